# revision 5
# baseline (speedup 1.0000x reference)
"""Trainium2 Bass kernel for nn_BottleneckS4D (8-core SPMD).

Strategy (self-contained, hardcoded):
  The reference is  u = x_flat @ Wb.T + bb  (256 x 150528 @ 150528 x 1280,
  770MB weight) followed by an S4D block whose output is only consumed at
  the LAST timestep (readout takes y[:, -1, :]), so the FFT convolution
  collapses to a per-channel dot product over time with the reversed S4D
  kernel, and everything downstream is tiny.

  Sharding: split the CONTRACTION dim D_IN=150528 across the 8 cores
  (18816 each). Each core streams its 96MB weight slice + 19MB x slice
  once (total HBM traffic = one pass over the data, the minimum), and
  computes a partial u^T (1280, 256) in PSUM with fp32r matmuls (full
  bf16-rate, ~1e-4 relative error). The S4D conv is linear in u, so each
  core reduces its partial u to a partial y_last (1280, 4) and a single
  tiny AllReduce (20KB) produces the exact y_last everywhere. GELU, the
  GLU 1x1 conv (sharded 160 channels/core + AllGather), and the readout
  MLP run on-device on every core; core 0's output is returned.
"""
import sys

sys.path.insert(0, "/opt/trn_rl_repo")
import numpy as np

B, T, H, N2 = 4, 64, 1280, 32
DIN = 224 * 224 * 3  # 150528
R_HID, NCLS = 64, 60
NCORES = 8
KS = DIN // NCORES   # 18816
KT = KS // 128       # 147
MT = H // 128        # 10
TOK = B * T          # 256
GO = H // NCORES     # 160 GLU output channels per core
HGO = GO // 2        # 80

_compiled = None


def _build():
    import concourse.bacc as bacc
    import concourse.mybir as mybir
    import concourse.tile as tile

    f32 = mybir.dt.float32
    f32r = mybir.dt.float32r
    AF = mybir.ActivationFunctionType
    OP = mybir.AluOpType
    RG = [list(range(NCORES))]

    nc = bacc.Bacc("TRN2", target_bir_lowering=False, debug=False,
                   num_devices=NCORES)

    d_xT = nc.dram_tensor("xT", [KS, TOK], f32r, kind="ExternalInput").ap()
    d_wT = nc.dram_tensor("wT", [KS, H], f32r, kind="ExternalInput").ap()
    d_bb = nc.dram_tensor("bb", [H, 1], f32, kind="ExternalInput").ap()
    d_logdt = nc.dram_tensor("logdt", [H, 1], f32, kind="ExternalInput").ap()
    d_logA = nc.dram_tensor("logA", [H, N2], f32, kind="ExternalInput").ap()
    d_C = nc.dram_tensor("Cmat", [H, N2], f32, kind="ExternalInput").ap()
    d_D = nc.dram_tensor("Dvec", [H, 1], f32, kind="ExternalInput").ap()
    d_rev = nc.dram_tensor("rev", [128, T * N2], f32, kind="ExternalInput").ap()
    d_wcT = nc.dram_tensor("wcT", [H, 4 * HGO], f32, kind="ExternalInput").ap()
    d_bc = nc.dram_tensor("bc", [4 * HGO, 1], f32, kind="ExternalInput").ap()
    d_w1T = nc.dram_tensor("w1T", [H, R_HID], f32, kind="ExternalInput").ap()
    d_b1 = nc.dram_tensor("b1", [R_HID, 1], f32, kind="ExternalInput").ap()
    d_w2T = nc.dram_tensor("w2T", [R_HID, NCLS], f32, kind="ExternalInput").ap()
    d_b2 = nc.dram_tensor("b2", [NCLS, 1], f32, kind="ExternalInput").ap()
    d_out = nc.dram_tensor("out", [NCLS, B], f32, kind="ExternalOutput").ap()

    with tile.TileContext(nc) as tc:
        with tc.tile_pool(name="cpool", bufs=1) as cpool, \
             tc.tile_pool(name="dram", bufs=1, space="DRAM") as dp:
            py_in = dp.tile([H, B], f32, tag="py_in")
            py_out = dp.tile([H, B], f32, tag="py_out", addr_space="Shared")
            glu_in = dp.tile([GO, B], f32, tag="glu_in")
            glu_out = dp.tile([H, B], f32, tag="glu_out", addr_space="Shared")

            # ---- Phase B: build reversed S4D kernels k_rev (per 128-chan tile)
            # k_rev[h, t] = 2 * sum_n C[h,n]*(exp(dtA[h,n])-1)/A[h,n]
            #                        * exp(dtA[h,n]*(63-t))
            rev_t = cpool.tile([128, T * N2], f32, tag="rev")
            nc.sync.dma_start(rev_t[:], d_rev)
            krev, bbs, Ds = [], [], []
            with tc.tile_pool(name="kb", bufs=2) as kb:
                for m in range(MT):
                    sl = slice(m * 128, (m + 1) * 128)
                    t_logdt = kb.tile([128, 1], f32, tag="logdt")
                    t_logA = kb.tile([128, N2], f32, tag="logA")
                    t_C = kb.tile([128, N2], f32, tag="C")
                    nc.sync.dma_start(t_logdt[:], d_logdt[sl, :])
                    nc.sync.dma_start(t_logA[:], d_logA[sl, :])
                    nc.sync.dma_start(t_C[:], d_C[sl, :])
                    t_bb = cpool.tile([128, 1], f32, tag=f"bb{m}")
                    t_D = cpool.tile([128, 1], f32, tag=f"D{m}")
                    nc.sync.dma_start(t_bb[:], d_bb[sl, :])
                    nc.sync.dma_start(t_D[:], d_D[sl, :])
                    bbs.append(t_bb)
                    Ds.append(t_D)
                    # bb/8: bias is added once globally via the AllReduce sum
                    nc.vector.tensor_scalar_mul(t_bb[:], t_bb[:], 1.0 / NCORES)

                    t_dt = kb.tile([128, 1], f32, tag="dt")
                    nc.scalar.activation(t_dt[:], t_logdt[:], AF.Exp)
                    negA = kb.tile([128, N2], f32, tag="negA")
                    nc.scalar.activation(negA[:], t_logA[:], AF.Exp)
                    dtA = kb.tile([128, N2], f32, tag="dtA")
                    # dtA = A*dt = -(negA*dt)
                    nc.vector.tensor_scalar(dtA[:], negA[:], t_dt[:], -1.0,
                                            OP.mult, OP.mult)
                    expdtA = kb.tile([128, N2], f32, tag="expdtA")
                    nc.scalar.activation(expdtA[:], dtA[:], AF.Exp)
                    recipA = kb.tile([128, N2], f32, tag="recipA")
                    nc.vector.reciprocal(recipA[:], negA[:])
                    # cb2 = 2*C*(exp(dtA)-1)/A = [ (expdtA-1)*(-2) ] * C * (1/negA)
                    cb2 = kb.tile([128, N2], f32, tag="cb2")
                    nc.vector.tensor_scalar(cb2[:], expdtA[:], 1.0, -2.0,
                                            OP.subtract, OP.mult)
                    nc.vector.tensor_mul(cb2[:], cb2[:], t_C[:])
                    nc.vector.tensor_mul(cb2[:], cb2[:], recipA[:])
                    # G[p, t, n] = rev[t] * dtA[p, n]
                    G = kb.tile([128, T * N2], f32, tag="G")
                    G3 = G[:].rearrange("p (t n) -> p t n", t=T)
                    nc.vector.tensor_tensor(
                        G3, rev_t[:].rearrange("p (t n) -> p t n", t=T),
                        dtA[:].unsqueeze(1).broadcast_to((128, T, N2)),
                        op=OP.mult)
                    expG = kb.tile([128, T * N2], f32, tag="expG")
                    nc.scalar.activation(expG[:], G[:], AF.Exp)
                    nc.vector.tensor_tensor(
                        expG[:].rearrange("p (t n) -> p t n", t=T),
                        expG[:].rearrange("p (t n) -> p t n", t=T),
                        cb2[:].unsqueeze(1).broadcast_to((128, T, N2)),
                        op=OP.mult)
                    kr = cpool.tile([128, T], f32, tag=f"krev{m}")
                    nc.vector.reduce_sum(
                        kr[:], expG[:].rearrange("p (t n) -> p t n", t=T),
                        axis=mybir.AxisListType.X)
                    krev.append(kr)

            # ---- Phase A: big matmul  u^T(partial) = wT_slice.T @ xT_slice
            with tc.tile_pool(name="wp", bufs=3) as wp, \
                 tc.tile_pool(name="xp", bufs=3) as xp, \
                 tc.tile_pool(name="psA", bufs=1, space="PSUM") as pA, \
                 tc.tile_pool(name="ev", bufs=2) as ev:
                from concourse.tile import add_dep_helper
                psu = [pA.tile([128, 512], f32, tag=f"u{j}", name=f"u{j}") for j in range(5)]
                mm_first, mm_last = {}, {}
                for k in range(KT):
                    wt = wp.tile([128, H], f32r, tag="wt")
                    xt = xp.tile([128, TOK], f32r, tag="xt")
                    nc.sync.dma_start(wt[:], d_wT[k * 128:(k + 1) * 128, :])
                    nc.sync.dma_start(xt[:], d_xT[k * 128:(k + 1) * 128, :])
                    for m in range(MT):
                        j, half = divmod(m, 2)
                        # two 256-wide accumulation groups share each 2KB PSUM
                        # bank: only the even half emits start (zeroing the
                        # whole bank region), only the odd half emits stop.
                        inst = nc.tensor.matmul(
                            psu[j][:, half * 256:(half + 1) * 256],
                            wt[:, m * 128:(m + 1) * 128], xt[:],
                            start=(k == 0 and half == 0),
                            stop=(k == KT - 1 and half == 1))
                        if k == 0:
                            mm_first[m] = inst
                        if k == KT - 1:
                            mm_last[m] = inst
                for j in range(5):
                    add_dep_helper(mm_first[2 * j + 1].ins, mm_first[2 * j].ins,
                                   reason="psum zero-region start order")
                    add_dep_helper(mm_last[2 * j + 1].ins, mm_last[2 * j].ins,
                                   reason="psum zero-region stop order")

                # ---- Phase C: u + bb/8, conv with k_rev, D-skip, partial y
                for m in range(MT):
                    j, half = divmod(m, 2)
                    u_sb = ev.tile([128, TOK], f32, tag="usb")
                    nc.scalar.activation(u_sb[:],
                                         psu[j][:, half * 256:(half + 1) * 256],
                                         AF.Identity, bias=bbs[m][:])
                    u3 = u_sb[:].rearrange("p (b t) -> p b t", b=B)
                    pr = ev.tile([128, TOK], f32, tag="pr")
                    nc.vector.tensor_tensor(
                        pr[:].rearrange("p (b t) -> p b t", b=B), u3,
                        krev[m][:].unsqueeze(1).broadcast_to((128, B, T)),
                        op=OP.mult)
                    y_m = ev.tile([128, B], f32, tag="ym")
                    nc.vector.reduce_sum(
                        y_m[:], pr[:].rearrange("p (b t) -> p b t", b=B),
                        axis=mybir.AxisListType.X)
                    dsk = ev.tile([128, B], f32, tag="dsk")
                    nc.vector.tensor_scalar(dsk[:], u3[:, :, T - 1], Ds[m][:],
                                            None, OP.mult)
                    nc.vector.tensor_add(y_m[:], y_m[:], dsk[:])
                    nc.sync.dma_start(py_in[m * 128:(m + 1) * 128, :], y_m[:])

                nc.gpsimd.collective_compute(
                    "AllReduce", OP.add, replica_groups=RG,
                    ins=[py_in.opt()], outs=[py_out.opt()])

            # ---- Phase D: GELU + GLU (sharded: 160 channels per core)
            with tc.tile_pool(name="de", bufs=1) as de, \
                 tc.tile_pool(name="psB", bufs=1, space="PSUM") as pB:
                yg = []
                for m in range(MT):
                    t_y = de.tile([128, B], f32, tag=f"yg{m}")
                    nc.sync.dma_start(t_y[:], py_out[m * 128:(m + 1) * 128, :])
                    nc.scalar.activation(t_y[:], t_y[:], AF.Gelu)
                    yg.append(t_y)
                z = [pB.tile([HGO, B], f32, tag=f"z{j}", name=f"z{j}") for j in range(4)]
                for k in range(MT):
                    wc = de.tile([128, 4 * HGO], f32, tag="wc")
                    nc.sync.dma_start(wc[:], d_wcT[k * 128:(k + 1) * 128, :])
                    for j in range(4):
                        nc.tensor.matmul(z[j][:], wc[:, j * HGO:(j + 1) * HGO],
                                         yg[k][:], start=(k == 0),
                                         stop=(k == MT - 1))
                bcg = []
                for j in range(4):
                    t = de.tile([HGO, 1], f32, tag=f"bc{j}")
                    nc.sync.dma_start(t[:], d_bc[j * HGO:(j + 1) * HGO, :])
                    bcg.append(t)
                a0 = de.tile([HGO, B], f32, tag="a0")
                a1 = de.tile([HGO, B], f32, tag="a1")
                s0 = de.tile([HGO, B], f32, tag="s0")
                s1 = de.tile([HGO, B], f32, tag="s1")
                nc.scalar.activation(a0[:], z[0][:], AF.Identity, bias=bcg[0][:])
                nc.scalar.activation(a1[:], z[1][:], AF.Identity, bias=bcg[1][:])
                nc.scalar.activation(s0[:], z[2][:], AF.Sigmoid, bias=bcg[2][:])
                nc.scalar.activation(s1[:], z[3][:], AF.Sigmoid, bias=bcg[3][:])
                g0 = de.tile([HGO, B], f32, tag="g0")
                g1 = de.tile([HGO, B], f32, tag="g1")
                nc.vector.tensor_mul(g0[:], a0[:], s0[:])
                nc.vector.tensor_mul(g1[:], a1[:], s1[:])
                nc.sync.dma_start(glu_in[0:HGO, :], g0[:])
                nc.sync.dma_start(glu_in[HGO:GO, :], g1[:])

                nc.gpsimd.collective_compute(
                    "AllGather", OP.bypass, replica_groups=RG,
                    ins=[glu_in.opt()], outs=[glu_out.opt()])

                # ---- Phase E: readout MLP
                ps_h = pB.tile([R_HID, B], f32, tag="ph")
                for k in range(MT):
                    gf = de.tile([128, B], f32, tag="gf")
                    nc.sync.dma_start(gf[:], glu_out[k * 128:(k + 1) * 128, :])
                    w1 = de.tile([128, R_HID], f32, tag="w1")
                    nc.sync.dma_start(w1[:], d_w1T[k * 128:(k + 1) * 128, :])
                    nc.tensor.matmul(ps_h[:], w1[:], gf[:], start=(k == 0),
                                     stop=(k == MT - 1))
                t_b1 = de.tile([R_HID, 1], f32, tag="b1")
                nc.sync.dma_start(t_b1[:], d_b1)
                h1 = de.tile([R_HID, B], f32, tag="h1")
                nc.scalar.activation(h1[:], ps_h[:], AF.Relu, bias=t_b1[:])
                w2 = de.tile([R_HID, NCLS], f32, tag="w2")
                nc.sync.dma_start(w2[:], d_w2T)
                ps_o = pB.tile([NCLS, B], f32, tag="po")
                nc.tensor.matmul(ps_o[:], w2[:], h1[:], start=True, stop=True)
                t_b2 = de.tile([NCLS, 1], f32, tag="b2")
                nc.sync.dma_start(t_b2[:], d_b2)
                o_sb = de.tile([NCLS, B], f32, tag="osb")
                nc.scalar.activation(o_sb[:], ps_o[:], AF.Identity,
                                     bias=t_b2[:])
                nc.sync.dma_start(d_out, o_sb[:])

    nc.compile()
    return nc


def _prep_inputs(inputs):
    x = np.asarray(inputs["x"], dtype=np.float32)
    Wb = np.asarray(inputs["Wb"], dtype=np.float32)
    bb = np.asarray(inputs["bb"], dtype=np.float32)
    log_dt = np.asarray(inputs["log_dt"], dtype=np.float32)
    C = np.asarray(inputs["C"], dtype=np.float32)
    logA = np.asarray(inputs["log_A_real"], dtype=np.float32)
    D = np.asarray(inputs["D"], dtype=np.float32)
    Wc = np.asarray(inputs["Wc"], dtype=np.float32)
    bc = np.asarray(inputs["bc"], dtype=np.float32)
    W1 = np.asarray(inputs["W1"], dtype=np.float32)
    b1 = np.asarray(inputs["b1"], dtype=np.float32)
    W2 = np.asarray(inputs["W2"], dtype=np.float32)
    b2 = np.asarray(inputs["b2"], dtype=np.float32)

    xT = np.ascontiguousarray(x.reshape(TOK, DIN).T)     # (DIN, 256)
    wT = np.ascontiguousarray(Wb.T)                      # (DIN, 1280)
    WcT = np.ascontiguousarray(Wc.T)                     # (1280, 2560)
    W1T = np.ascontiguousarray(W1.T)                     # (1280, 64)
    W2T = np.ascontiguousarray(W2.T)                     # (64, 60)
    rev = np.arange(T - 1, -1, -1, dtype=np.float32)     # 63 - t
    rev_full = np.ascontiguousarray(
        np.broadcast_to(np.repeat(rev, N2), (128, T * N2)))

    shared = {
        "bb": bb.reshape(H, 1), "logdt": log_dt.reshape(H, 1),
        "logA": logA, "Cmat": C, "Dvec": D.reshape(H, 1), "rev": rev_full,
        "w1T": W1T, "b1": b1.reshape(R_HID, 1),
        "w2T": W2T, "b2": b2.reshape(NCLS, 1),
    }
    in_maps = []
    for i in range(NCORES):
        klo = i * KS
        go = i * GO
        wcT_sl = np.ascontiguousarray(np.concatenate(
            [WcT[:, go:go + HGO], WcT[:, go + HGO:go + GO],
             WcT[:, H + go:H + go + HGO], WcT[:, H + go + HGO:H + go + GO]],
            axis=1))
        bc_sl = np.ascontiguousarray(np.concatenate(
            [bc[go:go + HGO], bc[go + HGO:go + GO],
             bc[H + go:H + go + HGO],
             bc[H + go + HGO:H + go + GO]]).reshape(4 * HGO, 1))
        in_maps.append({
            "xT": np.ascontiguousarray(xT[klo:klo + KS]),
            "wT": np.ascontiguousarray(wT[klo:klo + KS]),
            "wcT": wcT_sl, "bc": bc_sl, **shared,
        })
    return in_maps


def kernel(**inputs):
    global _compiled
    if _compiled is None:
        _compiled = _build()
    nc = _compiled
    in_maps = _prep_inputs(inputs)
    from concourse import bass_utils
    res = bass_utils.run_bass_kernel_spmd(nc, in_maps,
                                          core_ids=list(range(NCORES)))
    out = res.results[0]["out"]  # (NCLS, B)
    return np.ascontiguousarray(out.T).astype(np.float32)


# revision 7
# speedup vs baseline: 1.2439x; 1.2439x over previous
"""Trainium2 Bass kernel for nn_BottleneckS4D (8-core SPMD).

Strategy (self-contained, hardcoded):
  The reference is  u = x_flat @ Wb.T + bb  (256 x 150528 @ 150528 x 1280,
  770MB weight) followed by an S4D block whose output is only consumed at
  the LAST timestep (readout takes y[:, -1, :]), so the FFT convolution
  collapses to a per-channel dot product over time with the reversed S4D
  kernel, and everything downstream is tiny.

  Sharding: split the CONTRACTION dim D_IN=150528 across the 8 cores
  (18816 each). Each core streams its 96MB weight slice + 19MB x slice
  once (total HBM traffic = one pass over the data, the minimum), and
  computes a partial u^T (1280, 256) in PSUM with fp32r matmuls (full
  bf16-rate, ~1e-4 relative error). The S4D conv is linear in u, so each
  core reduces its partial u to a partial y_last (1280, 4) and a single
  tiny AllReduce (20KB) produces the exact y_last everywhere. GELU, the
  GLU 1x1 conv (sharded 160 channels/core + AllGather), and the readout
  MLP run on-device on every core; core 0's output is returned.

  Perf details: weights/x are host-repacked to partition-major layout
  (wTp[p, k, :] = wT[k*128+p, :]) so each DMA chunk moves CH k-tiles with
  one large contiguous descriptor per partition; wt streams on the sync
  HWDGE queue while xt + small tensors use the scalar HWDGE queue; a
  PE warmup burst (zeros matmuls into a scratch PSUM bank) lifts the HAM
  clock gate before the real matmuls arrive; a dummy 128B AllReduce early
  in the kernel absorbs the ~35us first-collective ncfw cost so the real
  AllReduce on the critical tail runs at the ~10us floor.
"""
import sys

sys.path.insert(0, "/opt/trn_rl_repo")
import numpy as np

B, T, H, N2 = 4, 64, 1280, 32
DIN = 224 * 224 * 3  # 150528
R_HID, NCLS = 64, 60
NCORES = 8
KS = DIN // NCORES   # 18816
KT = KS // 128       # 147
MT = H // 128        # 10
TOK = B * T          # 256
GO = H // NCORES     # 160 GLU output channels per core
HGO = GO // 2        # 80
CH = 7               # k-tiles per DMA chunk
NCHUNK = KT // CH    # 21

_compiled = None


def _build():
    import concourse.bacc as bacc
    import concourse.mybir as mybir
    import concourse.tile as tile
    from concourse.tile import add_dep_helper

    f32 = mybir.dt.float32
    f32r = mybir.dt.float32r
    AF = mybir.ActivationFunctionType
    OP = mybir.AluOpType
    RG = [list(range(NCORES))]

    nc = bacc.Bacc("TRN2", target_bir_lowering=False, debug=False,
                   num_devices=NCORES)

    d_xT = nc.dram_tensor("xT", [128, KT * TOK], f32r, kind="ExternalInput").ap()
    d_wT = nc.dram_tensor("wT", [128, KT * H], f32r, kind="ExternalInput").ap()
    d_bb = nc.dram_tensor("bb", [H, 1], f32, kind="ExternalInput").ap()
    d_logdt = nc.dram_tensor("logdt", [H, 1], f32, kind="ExternalInput").ap()
    d_logA = nc.dram_tensor("logA", [H, N2], f32, kind="ExternalInput").ap()
    d_C = nc.dram_tensor("Cmat", [H, N2], f32, kind="ExternalInput").ap()
    d_D = nc.dram_tensor("Dvec", [H, 1], f32, kind="ExternalInput").ap()
    d_rev = nc.dram_tensor("rev", [128, T * N2], f32, kind="ExternalInput").ap()
    d_wcT = nc.dram_tensor("wcT", [H, 4 * HGO], f32, kind="ExternalInput").ap()
    d_bc = nc.dram_tensor("bc", [4 * HGO, 1], f32, kind="ExternalInput").ap()
    d_w1T = nc.dram_tensor("w1T", [H, R_HID], f32, kind="ExternalInput").ap()
    d_b1 = nc.dram_tensor("b1", [R_HID, 1], f32, kind="ExternalInput").ap()
    d_w2T = nc.dram_tensor("w2T", [R_HID, NCLS], f32, kind="ExternalInput").ap()
    d_b2 = nc.dram_tensor("b2", [NCLS, 1], f32, kind="ExternalInput").ap()
    d_out = nc.dram_tensor("out", [NCLS, B], f32, kind="ExternalOutput").ap()

    with tile.TileContext(nc) as tc:
        with tc.tile_pool(name="cpool", bufs=1) as cpool, \
             tc.tile_pool(name="dram", bufs=1, space="DRAM") as dp:
            py_in = dp.tile([H, B], f32, tag="py_in")
            py_out = dp.tile([H, B], f32, tag="py_out", addr_space="Shared")
            glu_in = dp.tile([GO, B], f32, tag="glu_in")
            glu_out = dp.tile([H, B], f32, tag="glu_out", addr_space="Shared")

            # ---- collective warmup: tiny AllReduce absorbs ncfw first-call
            warm_in = dp.tile([NCORES, B], f32, tag="warm_in")
            warm_out = dp.tile([NCORES, B], f32, tag="warm_out",
                               addr_space="Shared")
            wz = cpool.tile([NCORES, B], f32, tag="wz")
            nc.vector.memset(wz[:], 0.0)
            nc.scalar.dma_start(warm_in[:, :], wz[:])
            nc.gpsimd.collective_compute(
                "AllReduce", OP.add, replica_groups=RG,
                ins=[warm_in.opt()], outs=[warm_out.opt()])

            # ---- Phase B: build reversed S4D kernels k_rev (per 128-chan tile)
            # k_rev[h, t] = 2 * sum_n C[h,n]*(exp(dtA[h,n])-1)/A[h,n]
            #                        * exp(dtA[h,n]*(63-t))
            rev_t = cpool.tile([128, T * N2], f32, tag="rev")
            nc.scalar.dma_start(rev_t[:], d_rev)
            krev, bbs, Ds = [], [], []
            with tc.tile_pool(name="kb", bufs=1) as kb:
                for m in range(MT):
                    sl = slice(m * 128, (m + 1) * 128)
                    t_logdt = kb.tile([128, 1], f32, tag="logdt")
                    t_logA = kb.tile([128, N2], f32, tag="logA")
                    t_C = kb.tile([128, N2], f32, tag="C")
                    nc.scalar.dma_start(t_logdt[:], d_logdt[sl, :])
                    nc.scalar.dma_start(t_logA[:], d_logA[sl, :])
                    nc.scalar.dma_start(t_C[:], d_C[sl, :])
                    t_bb = cpool.tile([128, 1], f32, tag=f"bb{m}")
                    t_D = cpool.tile([128, 1], f32, tag=f"D{m}")
                    nc.scalar.dma_start(t_bb[:], d_bb[sl, :])
                    nc.scalar.dma_start(t_D[:], d_D[sl, :])
                    bbs.append(t_bb)
                    Ds.append(t_D)
                    # bb/8: bias is added once globally via the AllReduce sum
                    nc.vector.tensor_scalar_mul(t_bb[:], t_bb[:], 1.0 / NCORES)

                    t_dt = kb.tile([128, 1], f32, tag="dt")
                    nc.scalar.activation(t_dt[:], t_logdt[:], AF.Exp)
                    negA = kb.tile([128, N2], f32, tag="negA")
                    nc.scalar.activation(negA[:], t_logA[:], AF.Exp)
                    dtA = kb.tile([128, N2], f32, tag="dtA")
                    # dtA = A*dt = -(negA*dt)
                    nc.vector.tensor_scalar(dtA[:], negA[:], t_dt[:], -1.0,
                                            OP.mult, OP.mult)
                    expdtA = kb.tile([128, N2], f32, tag="expdtA")
                    nc.scalar.activation(expdtA[:], dtA[:], AF.Exp)
                    recipA = kb.tile([128, N2], f32, tag="recipA")
                    nc.vector.reciprocal(recipA[:], negA[:])
                    # cb2 = 2*C*(exp(dtA)-1)/A = [ (expdtA-1)*(-2) ] * C * (1/negA)
                    cb2 = kb.tile([128, N2], f32, tag="cb2")
                    nc.vector.tensor_scalar(cb2[:], expdtA[:], 1.0, -2.0,
                                            OP.subtract, OP.mult)
                    nc.vector.tensor_mul(cb2[:], cb2[:], t_C[:])
                    nc.vector.tensor_mul(cb2[:], cb2[:], recipA[:])
                    # G[p, t, n] = rev[t] * dtA[p, n]
                    G = kb.tile([128, T * N2], f32, tag="G")
                    G3 = G[:].rearrange("p (t n) -> p t n", t=T)
                    nc.vector.tensor_tensor(
                        G3, rev_t[:].rearrange("p (t n) -> p t n", t=T),
                        dtA[:].unsqueeze(1).broadcast_to((128, T, N2)),
                        op=OP.mult)
                    expG = kb.tile([128, T * N2], f32, tag="expG")
                    nc.scalar.activation(expG[:], G[:], AF.Exp)
                    nc.vector.tensor_tensor(
                        expG[:].rearrange("p (t n) -> p t n", t=T),
                        expG[:].rearrange("p (t n) -> p t n", t=T),
                        cb2[:].unsqueeze(1).broadcast_to((128, T, N2)),
                        op=OP.mult)
                    kr = cpool.tile([128, T], f32, tag=f"krev{m}")
                    nc.vector.reduce_sum(
                        kr[:], expG[:].rearrange("p (t n) -> p t n", t=T),
                        axis=mybir.AxisListType.X)
                    krev.append(kr)

            # ---- Phase A: big matmul  u^T(partial) = wT_slice.T @ xT_slice
            with tc.tile_pool(name="wp", bufs=2) as wp, \
                 tc.tile_pool(name="xp", bufs=3) as xp, \
                 tc.tile_pool(name="psA", bufs=1, space="PSUM") as pA, \
                 tc.tile_pool(name="ev", bufs=2) as ev:
                psu = [pA.tile([128, 512], f32, tag=f"u{j}", name=f"u{j}")
                       for j in range(5)]

                # PE warmup: ~6us of zero matmuls into a scratch bank lifts
                # the HAM clock gate while the first weight chunk streams in.
                warm_ps = pA.tile([128, 512], f32, tag="warmps")
                warm_z = cpool.tile([128, 512], f32, tag="warmz")
                warm_w = cpool.tile([128, 128], f32r, tag="warmw")
                warm_x = cpool.tile([128, 512], f32r, tag="warmx")
                nc.vector.memset(warm_z[:], 0.0)
                nc.vector.tensor_copy(warm_w[:], warm_z[:, 0:128])
                nc.vector.tensor_copy(warm_x[:], warm_z[:])
                for _ in range(16):
                    nc.tensor.matmul(warm_ps[:], warm_w[:], warm_x[:],
                                     start=True, stop=True)

                mm_first, mm_last = {}, {}
                for kc in range(NCHUNK):
                    wt = wp.tile([128, CH * H], f32r, tag="wt")
                    xt = xp.tile([128, CH * TOK], f32r, tag="xt")
                    nc.sync.dma_start(
                        wt[:], d_wT[:, kc * CH * H:(kc + 1) * CH * H])
                    nc.scalar.dma_start(
                        xt[:], d_xT[:, kc * CH * TOK:(kc + 1) * CH * TOK])
                    for j_in in range(CH):
                        k = kc * CH + j_in
                        for m in range(MT):
                            j, half = divmod(m, 2)
                            # two 256-wide accumulation groups share each 2KB
                            # PSUM bank: only the even half emits start
                            # (zeroing the whole bank region), only the odd
                            # half emits stop.
                            inst = nc.tensor.matmul(
                                psu[j][:, half * 256:(half + 1) * 256],
                                wt[:, j_in * H + m * 128:
                                   j_in * H + (m + 1) * 128],
                                xt[:, j_in * TOK:(j_in + 1) * TOK],
                                start=(k == 0 and half == 0),
                                stop=(k == KT - 1 and half == 1))
                            if k == 0:
                                mm_first[m] = inst
                            if k == KT - 1:
                                mm_last[m] = inst
                for j in range(5):
                    add_dep_helper(mm_first[2 * j + 1].ins, mm_first[2 * j].ins,
                                   reason="psum zero-region start order")
                    add_dep_helper(mm_last[2 * j + 1].ins, mm_last[2 * j].ins,
                                   reason="psum zero-region stop order")

                # ---- Phase C: u + bb/8, conv with k_rev, D-skip, partial y
                for m in range(MT):
                    j, half = divmod(m, 2)
                    u_sb = ev.tile([128, TOK], f32, tag="usb")
                    nc.scalar.activation(u_sb[:],
                                         psu[j][:, half * 256:(half + 1) * 256],
                                         AF.Identity, bias=bbs[m][:])
                    u3 = u_sb[:].rearrange("p (b t) -> p b t", b=B)
                    pr = ev.tile([128, TOK], f32, tag="pr")
                    nc.vector.tensor_tensor(
                        pr[:].rearrange("p (b t) -> p b t", b=B), u3,
                        krev[m][:].unsqueeze(1).broadcast_to((128, B, T)),
                        op=OP.mult)
                    y_m = ev.tile([128, B], f32, tag="ym")
                    nc.vector.reduce_sum(
                        y_m[:], pr[:].rearrange("p (b t) -> p b t", b=B),
                        axis=mybir.AxisListType.X)
                    dsk = ev.tile([128, B], f32, tag="dsk")
                    nc.vector.tensor_scalar(dsk[:], u3[:, :, T - 1], Ds[m][:],
                                            None, OP.mult)
                    nc.vector.tensor_add(y_m[:], y_m[:], dsk[:])
                    nc.scalar.dma_start(py_in[m * 128:(m + 1) * 128, :], y_m[:])

                nc.gpsimd.collective_compute(
                    "AllReduce", OP.add, replica_groups=RG,
                    ins=[py_in.opt()], outs=[py_out.opt()])

            # ---- Phase D: GELU + GLU (sharded: 160 channels per core)
            with tc.tile_pool(name="de", bufs=1) as de, \
                 tc.tile_pool(name="psB", bufs=1, space="PSUM") as pB:
                yg = []
                for m in range(MT):
                    t_y = de.tile([128, B], f32, tag=f"yg{m}")
                    nc.scalar.dma_start(t_y[:], py_out[m * 128:(m + 1) * 128, :])
                    nc.scalar.activation(t_y[:], t_y[:], AF.Gelu)
                    yg.append(t_y)
                z = [pB.tile([HGO, B], f32, tag=f"z{j}", name=f"z{j}")
                     for j in range(4)]
                for k in range(MT):
                    wc = de.tile([128, 4 * HGO], f32, tag="wc", bufs=2)
                    nc.scalar.dma_start(wc[:], d_wcT[k * 128:(k + 1) * 128, :])
                    for j in range(4):
                        nc.tensor.matmul(z[j][:], wc[:, j * HGO:(j + 1) * HGO],
                                         yg[k][:], start=(k == 0),
                                         stop=(k == MT - 1))
                bcg = []
                for j in range(4):
                    t = de.tile([HGO, 1], f32, tag=f"bc{j}", name=f"bc{j}")
                    nc.scalar.dma_start(t[:], d_bc[j * HGO:(j + 1) * HGO, :])
                    bcg.append(t)
                a0 = de.tile([HGO, B], f32, tag="a0")
                a1 = de.tile([HGO, B], f32, tag="a1")
                s0 = de.tile([HGO, B], f32, tag="s0")
                s1 = de.tile([HGO, B], f32, tag="s1")
                nc.scalar.activation(a0[:], z[0][:], AF.Identity, bias=bcg[0][:])
                nc.scalar.activation(a1[:], z[1][:], AF.Identity, bias=bcg[1][:])
                nc.scalar.activation(s0[:], z[2][:], AF.Sigmoid, bias=bcg[2][:])
                nc.scalar.activation(s1[:], z[3][:], AF.Sigmoid, bias=bcg[3][:])
                g0 = de.tile([HGO, B], f32, tag="g0")
                g1 = de.tile([HGO, B], f32, tag="g1")
                nc.vector.tensor_mul(g0[:], a0[:], s0[:])
                nc.vector.tensor_mul(g1[:], a1[:], s1[:])
                nc.scalar.dma_start(glu_in[0:HGO, :], g0[:])
                nc.scalar.dma_start(glu_in[HGO:GO, :], g1[:])

                nc.gpsimd.collective_compute(
                    "AllGather", OP.bypass, replica_groups=RG,
                    ins=[glu_in.opt()], outs=[glu_out.opt()])

                # ---- Phase E: readout MLP
                ps_h = pB.tile([R_HID, B], f32, tag="ph")
                for k in range(MT):
                    gf = de.tile([128, B], f32, tag="gf", bufs=2)
                    nc.scalar.dma_start(gf[:], glu_out[k * 128:(k + 1) * 128, :])
                    w1 = de.tile([128, R_HID], f32, tag="w1", bufs=2)
                    nc.scalar.dma_start(w1[:], d_w1T[k * 128:(k + 1) * 128, :])
                    nc.tensor.matmul(ps_h[:], w1[:], gf[:], start=(k == 0),
                                     stop=(k == MT - 1))
                t_b1 = de.tile([R_HID, 1], f32, tag="b1")
                nc.scalar.dma_start(t_b1[:], d_b1)
                h1 = de.tile([R_HID, B], f32, tag="h1")
                nc.scalar.activation(h1[:], ps_h[:], AF.Relu, bias=t_b1[:])
                w2 = de.tile([R_HID, NCLS], f32, tag="w2")
                nc.scalar.dma_start(w2[:], d_w2T)
                ps_o = pB.tile([NCLS, B], f32, tag="po")
                nc.tensor.matmul(ps_o[:], w2[:], h1[:], start=True, stop=True)
                t_b2 = de.tile([NCLS, 1], f32, tag="b2")
                nc.scalar.dma_start(t_b2[:], d_b2)
                o_sb = de.tile([NCLS, B], f32, tag="osb")
                nc.scalar.activation(o_sb[:], ps_o[:], AF.Identity,
                                     bias=t_b2[:])
                nc.scalar.dma_start(d_out, o_sb[:])

    nc.compile()
    return nc


def _prep_inputs(inputs):
    x = np.asarray(inputs["x"], dtype=np.float32)
    Wb = np.asarray(inputs["Wb"], dtype=np.float32)
    bb = np.asarray(inputs["bb"], dtype=np.float32)
    log_dt = np.asarray(inputs["log_dt"], dtype=np.float32)
    C = np.asarray(inputs["C"], dtype=np.float32)
    logA = np.asarray(inputs["log_A_real"], dtype=np.float32)
    D = np.asarray(inputs["D"], dtype=np.float32)
    Wc = np.asarray(inputs["Wc"], dtype=np.float32)
    bc = np.asarray(inputs["bc"], dtype=np.float32)
    W1 = np.asarray(inputs["W1"], dtype=np.float32)
    b1 = np.asarray(inputs["b1"], dtype=np.float32)
    W2 = np.asarray(inputs["W2"], dtype=np.float32)
    b2 = np.asarray(inputs["b2"], dtype=np.float32)

    xT = np.ascontiguousarray(x.reshape(TOK, DIN).T)     # (DIN, 256)
    wT = np.ascontiguousarray(Wb.T)                      # (DIN, 1280)
    WcT = np.ascontiguousarray(Wc.T)                     # (1280, 2560)
    W1T = np.ascontiguousarray(W1.T)                     # (1280, 64)
    W2T = np.ascontiguousarray(W2.T)                     # (64, 60)
    rev = np.arange(T - 1, -1, -1, dtype=np.float32)     # 63 - t
    rev_full = np.ascontiguousarray(
        np.broadcast_to(np.repeat(rev, N2), (128, T * N2)))

    shared = {
        "bb": bb.reshape(H, 1), "logdt": log_dt.reshape(H, 1),
        "logA": logA, "Cmat": C, "Dvec": D.reshape(H, 1), "rev": rev_full,
        "w1T": W1T, "b1": b1.reshape(R_HID, 1),
        "w2T": W2T, "b2": b2.reshape(NCLS, 1),
    }
    in_maps = []
    for i in range(NCORES):
        klo = i * KS
        go = i * GO
        # partition-major repack: arr_p[p, k, :] = arr[k*128+p, :] so each
        # DMA chunk reads one large contiguous block per partition
        wTp = np.ascontiguousarray(
            wT[klo:klo + KS].reshape(KT, 128, H).transpose(1, 0, 2)
        ).reshape(128, KT * H)
        xTp = np.ascontiguousarray(
            xT[klo:klo + KS].reshape(KT, 128, TOK).transpose(1, 0, 2)
        ).reshape(128, KT * TOK)
        wcT_sl = np.ascontiguousarray(np.concatenate(
            [WcT[:, go:go + HGO], WcT[:, go + HGO:go + GO],
             WcT[:, H + go:H + go + HGO], WcT[:, H + go + HGO:H + go + GO]],
            axis=1))
        bc_sl = np.ascontiguousarray(np.concatenate(
            [bc[go:go + HGO], bc[go + HGO:go + GO],
             bc[H + go:H + go + HGO],
             bc[H + go + HGO:H + go + GO]]).reshape(4 * HGO, 1))
        in_maps.append({
            "xT": xTp, "wT": wTp,
            "wcT": wcT_sl, "bc": bc_sl, **shared,
        })
    return in_maps


def kernel(**inputs):
    global _compiled
    if _compiled is None:
        _compiled = _build()
    nc = _compiled
    in_maps = _prep_inputs(inputs)
    from concourse import bass_utils
    res = bass_utils.run_bass_kernel_spmd(nc, in_maps,
                                          core_ids=list(range(NCORES)))
    out = res.results[0]["out"]  # (NCLS, B)
    return np.ascontiguousarray(out.T).astype(np.float32)


# revision 8
# speedup vs baseline: 1.2786x; 1.0279x over previous
"""Trainium2 Bass kernel for nn_BottleneckS4D (8-core SPMD).

Strategy (self-contained, hardcoded):
  The reference is  u = x_flat @ Wb.T + bb  (256 x 150528 @ 150528 x 1280,
  770MB weight) followed by an S4D block whose output is only consumed at
  the LAST timestep (readout takes y[:, -1, :]), so the FFT convolution
  collapses to a per-channel dot product over time with the reversed S4D
  kernel, and everything downstream is tiny.

  Sharding: split the CONTRACTION dim D_IN=150528 across the 8 cores
  (18816 each). Each core streams its 96MB weight slice + 19MB x slice
  once (total HBM traffic = one pass over the data, the minimum), and
  computes a partial u^T (1280, 256) in PSUM with fp32r matmuls (full
  bf16-rate, ~1e-4 relative error). The S4D conv is linear in u, so each
  core reduces its partial u to a partial y_last (1280, 4) and a single
  tiny AllReduce (20KB) produces the exact y_last everywhere. GELU, the
  GLU 1x1 conv (sharded 160 channels/core + AllGather), and the readout
  MLP run on-device on every core; core 0's output is returned.

  Perf details: weights/x are host-repacked to partition-major layout
  (wTp[p, k, :] = wT[k*128+p, :]) so each DMA chunk moves CH k-tiles with
  one large contiguous descriptor per partition; wt streams on the sync
  HWDGE queue while xt + small tensors use the scalar HWDGE queue; a
  PE warmup burst (zeros matmuls into a scratch PSUM bank) lifts the HAM
  clock gate before the real matmuls arrive; a dummy 128B AllReduce early
  in the kernel absorbs the ~35us first-collective ncfw cost so the real
  AllReduce on the critical tail runs at the ~10us floor.
"""
import sys

sys.path.insert(0, "/opt/trn_rl_repo")
import numpy as np

B, T, H, N2 = 4, 64, 1280, 32
DIN = 224 * 224 * 3  # 150528
R_HID, NCLS = 64, 60
NCORES = 8
KS = DIN // NCORES   # 18816
KT = KS // 128       # 147
MT = H // 128        # 10
TOK = B * T          # 256
GO = H // NCORES     # 160 GLU output channels per core
HGO = GO // 2        # 80
CH = 3               # k-tiles per DMA chunk
NCHUNK = KT // CH    # 49

_compiled = None


def _build():
    import concourse.bacc as bacc
    import concourse.mybir as mybir
    import concourse.tile as tile
    from concourse.tile import add_dep_helper

    f32 = mybir.dt.float32
    f32r = mybir.dt.float32r
    AF = mybir.ActivationFunctionType
    OP = mybir.AluOpType
    RG = [list(range(NCORES))]

    nc = bacc.Bacc("TRN2", target_bir_lowering=False, debug=False,
                   num_devices=NCORES)

    d_xT = nc.dram_tensor("xT", [128, KT * TOK], f32r, kind="ExternalInput").ap()
    d_wT = nc.dram_tensor("wT", [128, KT * H], f32r, kind="ExternalInput").ap()
    d_bb = nc.dram_tensor("bb", [H, 1], f32, kind="ExternalInput").ap()
    d_logdt = nc.dram_tensor("logdt", [H, 1], f32, kind="ExternalInput").ap()
    d_logA = nc.dram_tensor("logA", [H, N2], f32, kind="ExternalInput").ap()
    d_C = nc.dram_tensor("Cmat", [H, N2], f32, kind="ExternalInput").ap()
    d_D = nc.dram_tensor("Dvec", [H, 1], f32, kind="ExternalInput").ap()
    d_rev = nc.dram_tensor("rev", [128, T * N2], f32, kind="ExternalInput").ap()
    d_wcT = nc.dram_tensor("wcT", [H, 4 * HGO], f32r, kind="ExternalInput").ap()
    d_bc = nc.dram_tensor("bc", [4 * HGO, 1], f32, kind="ExternalInput").ap()
    d_w1T = nc.dram_tensor("w1T", [H, R_HID], f32r, kind="ExternalInput").ap()
    d_b1 = nc.dram_tensor("b1", [R_HID, 1], f32, kind="ExternalInput").ap()
    d_w2T = nc.dram_tensor("w2T", [R_HID, NCLS], f32r, kind="ExternalInput").ap()
    d_b2 = nc.dram_tensor("b2", [NCLS, 1], f32, kind="ExternalInput").ap()
    d_out = nc.dram_tensor("out", [NCLS, B], f32, kind="ExternalOutput").ap()

    with tile.TileContext(nc) as tc:
        with tc.tile_pool(name="cpool", bufs=1) as cpool, \
             tc.tile_pool(name="dram", bufs=1, space="DRAM") as dp:
            py_in = dp.tile([H, B], f32, tag="py_in")
            py_out = dp.tile([H, B], f32, tag="py_out", addr_space="Shared")
            glu_in = dp.tile([GO, B], f32r, tag="glu_in")
            glu_out = dp.tile([H, B], f32r, tag="glu_out", addr_space="Shared")

            # ---- collective warmup: tiny AllReduce absorbs ncfw first-call
            warm_in = dp.tile([NCORES, B], f32, tag="warm_in")
            warm_out = dp.tile([NCORES, B], f32, tag="warm_out",
                               addr_space="Shared")
            wz = cpool.tile([NCORES, B], f32, tag="wz")
            nc.vector.memset(wz[:], 0.0)
            nc.scalar.dma_start(warm_in[:, :], wz[:])
            nc.gpsimd.collective_compute(
                "AllReduce", OP.add, replica_groups=RG,
                ins=[warm_in.opt()], outs=[warm_out.opt()])

            # ---- Phase B: build reversed S4D kernels k_rev (per 128-chan tile)
            # k_rev[h, t] = 2 * sum_n C[h,n]*(exp(dtA[h,n])-1)/A[h,n]
            #                        * exp(dtA[h,n]*(63-t))
            rev_t = cpool.tile([128, T * N2], f32, tag="rev")
            nc.scalar.dma_start(rev_t[:], d_rev)
            krev, bbs, Ds = [], [], []
            with tc.tile_pool(name="kb", bufs=1) as kb:
                for m in range(MT):
                    sl = slice(m * 128, (m + 1) * 128)
                    t_logdt = kb.tile([128, 1], f32, tag="logdt")
                    t_logA = kb.tile([128, N2], f32, tag="logA")
                    t_C = kb.tile([128, N2], f32, tag="C")
                    nc.scalar.dma_start(t_logdt[:], d_logdt[sl, :])
                    nc.scalar.dma_start(t_logA[:], d_logA[sl, :])
                    nc.scalar.dma_start(t_C[:], d_C[sl, :])
                    t_bb = cpool.tile([128, 1], f32, tag=f"bb{m}")
                    t_D = cpool.tile([128, 1], f32, tag=f"D{m}")
                    nc.scalar.dma_start(t_bb[:], d_bb[sl, :])
                    nc.scalar.dma_start(t_D[:], d_D[sl, :])
                    bbs.append(t_bb)
                    Ds.append(t_D)
                    # bb/8: bias is added once globally via the AllReduce sum
                    nc.vector.tensor_scalar_mul(t_bb[:], t_bb[:], 1.0 / NCORES)

                    t_dt = kb.tile([128, 1], f32, tag="dt")
                    nc.scalar.activation(t_dt[:], t_logdt[:], AF.Exp)
                    negA = kb.tile([128, N2], f32, tag="negA")
                    nc.scalar.activation(negA[:], t_logA[:], AF.Exp)
                    dtA = kb.tile([128, N2], f32, tag="dtA")
                    # dtA = A*dt = -(negA*dt)
                    nc.vector.tensor_scalar(dtA[:], negA[:], t_dt[:], -1.0,
                                            OP.mult, OP.mult)
                    expdtA = kb.tile([128, N2], f32, tag="expdtA")
                    nc.scalar.activation(expdtA[:], dtA[:], AF.Exp)
                    recipA = kb.tile([128, N2], f32, tag="recipA")
                    nc.vector.reciprocal(recipA[:], negA[:])
                    # cb2 = 2*C*(exp(dtA)-1)/A = [ (expdtA-1)*(-2) ] * C * (1/negA)
                    cb2 = kb.tile([128, N2], f32, tag="cb2")
                    nc.vector.tensor_scalar(cb2[:], expdtA[:], 1.0, -2.0,
                                            OP.subtract, OP.mult)
                    nc.vector.tensor_mul(cb2[:], cb2[:], t_C[:])
                    nc.vector.tensor_mul(cb2[:], cb2[:], recipA[:])
                    # G[p, t, n] = rev[t] * dtA[p, n]
                    G = kb.tile([128, T * N2], f32, tag="G")
                    G3 = G[:].rearrange("p (t n) -> p t n", t=T)
                    nc.vector.tensor_tensor(
                        G3, rev_t[:].rearrange("p (t n) -> p t n", t=T),
                        dtA[:].unsqueeze(1).broadcast_to((128, T, N2)),
                        op=OP.mult)
                    expG = kb.tile([128, T * N2], f32, tag="expG")
                    nc.scalar.activation(expG[:], G[:], AF.Exp)
                    nc.vector.tensor_tensor(
                        expG[:].rearrange("p (t n) -> p t n", t=T),
                        expG[:].rearrange("p (t n) -> p t n", t=T),
                        cb2[:].unsqueeze(1).broadcast_to((128, T, N2)),
                        op=OP.mult)
                    kr = cpool.tile([128, T], f32, tag=f"krev{m}")
                    nc.vector.reduce_sum(
                        kr[:], expG[:].rearrange("p (t n) -> p t n", t=T),
                        axis=mybir.AxisListType.X)
                    krev.append(kr)

            # ---- preload epilogue weights (overlaps the big matmul stream)
            wcs, w1s = [], []
            for k in range(MT):
                t_wc = cpool.tile([128, 4 * HGO], f32r, tag=f"wc{k}",
                                  name=f"wc{k}")
                nc.sync.dma_start(t_wc[:], d_wcT[k * 128:(k + 1) * 128, :])
                wcs.append(t_wc)
                t_w1 = cpool.tile([128, R_HID], f32r, tag=f"w1_{k}",
                                  name=f"w1_{k}")
                nc.sync.dma_start(t_w1[:], d_w1T[k * 128:(k + 1) * 128, :])
                w1s.append(t_w1)
            bcg = []
            for j in range(4):
                t_bc = cpool.tile([HGO, 1], f32, tag=f"bc{j}", name=f"bc{j}")
                nc.scalar.dma_start(t_bc[:], d_bc[j * HGO:(j + 1) * HGO, :])
                bcg.append(t_bc)
            w2 = cpool.tile([R_HID, NCLS], f32r, tag="w2")
            nc.scalar.dma_start(w2[:], d_w2T)
            t_b1 = cpool.tile([R_HID, 1], f32, tag="b1")
            nc.scalar.dma_start(t_b1[:], d_b1)
            t_b2 = cpool.tile([NCLS, 1], f32, tag="b2")
            nc.scalar.dma_start(t_b2[:], d_b2)

            # ---- Phase A: big matmul  u^T(partial) = wT_slice.T @ xT_slice
            with tc.tile_pool(name="wp", bufs=4) as wp, \
                 tc.tile_pool(name="xp", bufs=4) as xp, \
                 tc.tile_pool(name="psA", bufs=1, space="PSUM") as pA, \
                 tc.tile_pool(name="ev", bufs=2) as ev:
                psu = [pA.tile([128, 512], f32, tag=f"u{j}", name=f"u{j}")
                       for j in range(5)]

                # PE warmup: ~6us of zero matmuls into a scratch bank lifts
                # the HAM clock gate while the first weight chunk streams in.
                warm_ps = pA.tile([128, 512], f32, tag="warmps")
                warm_z = cpool.tile([128, 512], f32, tag="warmz")
                warm_w = cpool.tile([128, 128], f32r, tag="warmw")
                warm_x = cpool.tile([128, 512], f32r, tag="warmx")
                nc.vector.memset(warm_z[:], 0.0)
                nc.vector.tensor_copy(warm_w[:], warm_z[:, 0:128])
                nc.vector.tensor_copy(warm_x[:], warm_z[:])
                for _ in range(16):
                    nc.tensor.matmul(warm_ps[:], warm_w[:], warm_x[:],
                                     start=True, stop=True)

                mm_first, mm_last = {}, {}
                for kc in range(NCHUNK):
                    wt = wp.tile([128, CH * H], f32r, tag="wt")
                    xt = xp.tile([128, CH * TOK], f32r, tag="xt")
                    wq = nc.sync if kc % 2 == 0 else nc.scalar
                    xq = nc.scalar if kc % 2 == 0 else nc.sync
                    wq.dma_start(
                        wt[:], d_wT[:, kc * CH * H:(kc + 1) * CH * H])
                    xq.dma_start(
                        xt[:], d_xT[:, kc * CH * TOK:(kc + 1) * CH * TOK])
                    for j_in in range(CH):
                        k = kc * CH + j_in
                        for m in range(MT):
                            j, half = divmod(m, 2)
                            # two 256-wide accumulation groups share each 2KB
                            # PSUM bank: only the even half emits start
                            # (zeroing the whole bank region), only the odd
                            # half emits stop.
                            inst = nc.tensor.matmul(
                                psu[j][:, half * 256:(half + 1) * 256],
                                wt[:, j_in * H + m * 128:
                                   j_in * H + (m + 1) * 128],
                                xt[:, j_in * TOK:(j_in + 1) * TOK],
                                start=(k == 0 and half == 0),
                                stop=(k == KT - 1 and half == 1))
                            if k == 0:
                                mm_first[m] = inst
                            if k == KT - 1:
                                mm_last[m] = inst
                for j in range(5):
                    add_dep_helper(mm_first[2 * j + 1].ins, mm_first[2 * j].ins,
                                   reason="psum zero-region start order")
                    add_dep_helper(mm_last[2 * j + 1].ins, mm_last[2 * j].ins,
                                   reason="psum zero-region stop order")

                # ---- Phase C: u + bb/8, conv with k_rev, D-skip, partial y
                for m in range(MT):
                    j, half = divmod(m, 2)
                    u_sb = ev.tile([128, TOK], f32, tag="usb")
                    nc.scalar.activation(u_sb[:],
                                         psu[j][:, half * 256:(half + 1) * 256],
                                         AF.Identity, bias=bbs[m][:])
                    u3 = u_sb[:].rearrange("p (b t) -> p b t", b=B)
                    pr = ev.tile([128, TOK], f32, tag="pr")
                    nc.vector.tensor_tensor(
                        pr[:].rearrange("p (b t) -> p b t", b=B), u3,
                        krev[m][:].unsqueeze(1).broadcast_to((128, B, T)),
                        op=OP.mult)
                    y_m = ev.tile([128, B], f32, tag="ym")
                    nc.vector.reduce_sum(
                        y_m[:], pr[:].rearrange("p (b t) -> p b t", b=B),
                        axis=mybir.AxisListType.X)
                    dsk = ev.tile([128, B], f32, tag="dsk")
                    nc.vector.tensor_scalar(dsk[:], u3[:, :, T - 1], Ds[m][:],
                                            None, OP.mult)
                    nc.vector.tensor_add(y_m[:], y_m[:], dsk[:])
                    nc.scalar.dma_start(py_in[m * 128:(m + 1) * 128, :], y_m[:])

                nc.gpsimd.collective_compute(
                    "AllReduce", OP.add, replica_groups=RG,
                    ins=[py_in.opt()], outs=[py_out.opt()])

            # ---- Phase D: GELU + GLU (sharded: 160 channels per core)
            with tc.tile_pool(name="de", bufs=1) as de, \
                 tc.tile_pool(name="psB", bufs=1, space="PSUM") as pB:
                # one DMA gathers all of y_last: yg_all[p, m, b] = py_out[m*128+p, b]
                yg_raw = de.tile([128, MT * B], f32, tag="ygraw")
                src_y = py_out.rearrange("(m p) b -> p m b", p=128)
                nc.scalar.dma_start(
                    yg_raw[:].rearrange("p (m b) -> p m b", m=MT), src_y)
                yg_all = de.tile([128, MT * B], f32r, tag="ygall")
                nc.scalar.activation(yg_all[:], yg_raw[:], AF.Gelu)
                yg = [yg_all[:, m * B:(m + 1) * B] for m in range(MT)]
                z = [pB.tile([HGO, B], f32, tag=f"z{j}", name=f"z{j}")
                     for j in range(4)]
                for k in range(MT):
                    for j in range(4):
                        nc.tensor.matmul(z[j][:],
                                         wcs[k][:, j * HGO:(j + 1) * HGO],
                                         yg[k], start=(k == 0),
                                         stop=(k == MT - 1))
                a0 = de.tile([HGO, B], f32, tag="a0")
                a1 = de.tile([HGO, B], f32, tag="a1")
                s0 = de.tile([HGO, B], f32, tag="s0")
                s1 = de.tile([HGO, B], f32, tag="s1")
                nc.scalar.activation(a0[:], z[0][:], AF.Identity, bias=bcg[0][:])
                nc.scalar.activation(a1[:], z[1][:], AF.Identity, bias=bcg[1][:])
                nc.scalar.activation(s0[:], z[2][:], AF.Sigmoid, bias=bcg[2][:])
                nc.scalar.activation(s1[:], z[3][:], AF.Sigmoid, bias=bcg[3][:])
                g0 = de.tile([HGO, B], f32r, tag="g0")
                g1 = de.tile([HGO, B], f32r, tag="g1")
                nc.vector.tensor_mul(g0[:], a0[:], s0[:])
                nc.vector.tensor_mul(g1[:], a1[:], s1[:])
                nc.scalar.dma_start(glu_in[0:HGO, :], g0[:])
                nc.scalar.dma_start(glu_in[HGO:GO, :], g1[:])

                nc.gpsimd.collective_compute(
                    "AllGather", OP.bypass, replica_groups=RG,
                    ins=[glu_in.opt()], outs=[glu_out.opt()])

                # ---- Phase E: readout MLP
                ps_h = pB.tile([R_HID, B], f32, tag="ph")
                gf_all = de.tile([128, MT * B], f32r, tag="gfall")
                src_g = glu_out.rearrange("(m p) b -> p m b", p=128)
                nc.scalar.dma_start(
                    gf_all[:].rearrange("p (m b) -> p m b", m=MT), src_g)
                for k in range(MT):
                    nc.tensor.matmul(ps_h[:], w1s[k][:],
                                     gf_all[:, k * B:(k + 1) * B],
                                     start=(k == 0), stop=(k == MT - 1))
                h1 = de.tile([R_HID, B], f32r, tag="h1")
                nc.scalar.activation(h1[:], ps_h[:], AF.Relu, bias=t_b1[:])
                ps_o = pB.tile([NCLS, B], f32, tag="po")
                nc.tensor.matmul(ps_o[:], w2[:], h1[:], start=True, stop=True)
                o_sb = de.tile([NCLS, B], f32, tag="osb")
                nc.scalar.activation(o_sb[:], ps_o[:], AF.Identity,
                                     bias=t_b2[:])
                nc.scalar.dma_start(d_out, o_sb[:])

    nc.compile()
    return nc


def _prep_inputs(inputs):
    x = np.asarray(inputs["x"], dtype=np.float32)
    Wb = np.asarray(inputs["Wb"], dtype=np.float32)
    bb = np.asarray(inputs["bb"], dtype=np.float32)
    log_dt = np.asarray(inputs["log_dt"], dtype=np.float32)
    C = np.asarray(inputs["C"], dtype=np.float32)
    logA = np.asarray(inputs["log_A_real"], dtype=np.float32)
    D = np.asarray(inputs["D"], dtype=np.float32)
    Wc = np.asarray(inputs["Wc"], dtype=np.float32)
    bc = np.asarray(inputs["bc"], dtype=np.float32)
    W1 = np.asarray(inputs["W1"], dtype=np.float32)
    b1 = np.asarray(inputs["b1"], dtype=np.float32)
    W2 = np.asarray(inputs["W2"], dtype=np.float32)
    b2 = np.asarray(inputs["b2"], dtype=np.float32)

    xT = np.ascontiguousarray(x.reshape(TOK, DIN).T)     # (DIN, 256)
    wT = np.ascontiguousarray(Wb.T)                      # (DIN, 1280)
    WcT = np.ascontiguousarray(Wc.T)                     # (1280, 2560)
    W1T = np.ascontiguousarray(W1.T)                     # (1280, 64)
    W2T = np.ascontiguousarray(W2.T)                     # (64, 60)
    rev = np.arange(T - 1, -1, -1, dtype=np.float32)     # 63 - t
    rev_full = np.ascontiguousarray(
        np.broadcast_to(np.repeat(rev, N2), (128, T * N2)))

    shared = {
        "bb": bb.reshape(H, 1), "logdt": log_dt.reshape(H, 1),
        "logA": logA, "Cmat": C, "Dvec": D.reshape(H, 1), "rev": rev_full,
        "w1T": W1T, "b1": b1.reshape(R_HID, 1),
        "w2T": W2T, "b2": b2.reshape(NCLS, 1),
    }
    in_maps = []
    for i in range(NCORES):
        klo = i * KS
        go = i * GO
        # partition-major repack: arr_p[p, k, :] = arr[k*128+p, :] so each
        # DMA chunk reads one large contiguous block per partition
        wTp = np.ascontiguousarray(
            wT[klo:klo + KS].reshape(KT, 128, H).transpose(1, 0, 2)
        ).reshape(128, KT * H)
        xTp = np.ascontiguousarray(
            xT[klo:klo + KS].reshape(KT, 128, TOK).transpose(1, 0, 2)
        ).reshape(128, KT * TOK)
        wcT_sl = np.ascontiguousarray(np.concatenate(
            [WcT[:, go:go + HGO], WcT[:, go + HGO:go + GO],
             WcT[:, H + go:H + go + HGO], WcT[:, H + go + HGO:H + go + GO]],
            axis=1))
        bc_sl = np.ascontiguousarray(np.concatenate(
            [bc[go:go + HGO], bc[go + HGO:go + GO],
             bc[H + go:H + go + HGO],
             bc[H + go + HGO:H + go + GO]]).reshape(4 * HGO, 1))
        in_maps.append({
            "xT": xTp, "wT": wTp,
            "wcT": wcT_sl, "bc": bc_sl, **shared,
        })
    return in_maps


def kernel(**inputs):
    global _compiled
    if _compiled is None:
        _compiled = _build()
    nc = _compiled
    in_maps = _prep_inputs(inputs)
    from concourse import bass_utils
    res = bass_utils.run_bass_kernel_spmd(nc, in_maps,
                                          core_ids=list(range(NCORES)))
    out = res.results[0]["out"]  # (NCLS, B)
    return np.ascontiguousarray(out.T).astype(np.float32)
